# revision 5
# baseline (speedup 1.0000x reference)
"""Trainium2 Bass kernel for fused QKV-projection + multi-head attention.

Problem: x[2,2048,1024] @ W_qkv[1024,3072] + b -> split q/k/v -> 16 heads of
dim 64 -> softmax(q k^T / 8) v -> [2,2048,1024].

Sharding (8 cores): data-parallel over batch (2) x tensor-parallel over head
groups (4 heads per core).  Each core computes a disjoint output slice
[2048, 256]; no collectives are needed.

Design notes:
- Matmul operands are fp16 (fp32 PSUM accumulation): full-rate PE with
  overlapped weight loads.  fp8 double-row was evaluated and rejected: the
  harness tolerance (2e-2 vs max|y|~0.13) leaves no room for e4m3's ~3 pct
  element error (measured 2.3e-2 end-to-end).
- Inputs are pre-arranged on the host so every DMA moves >=2KB contiguous
  per-partition lines: x arrives t-block-major ([tb, p, cb, t]), w arrives
  column-group-major ([p, cb*col]).  v_proj(tb) starts as soon as its 256KB
  block lands (~1us) instead of waiting ~14us for the whole 4MiB transfer.
- The attention phase is ACT(exp)-paced (~2.1us per kb stage vs ~1.9us of
  PE work), so the remaining projection work is drip-fed as "filler" matmul
  groups, one per kb stage.  Fillers always use the S1 PSUM slot and are
  issued at the top of the stage, ordered [filler, AV(kb-1), scores(s1),
  scores(s0)], so the filler's slot-wait lands on a stage-old exp and the
  filler's DVE bias-read hides behind the AV matmuls.
- kT is stored packed per head-pair on the partition axis; qT per head is
  zero-padded to 128 partitions so a full-128 matmul against the pair tile
  selects a single head's scores.  scoresT [k, q] layout keeps softmax's
  reduction on the PE (ones-column appended to V: [E^T V | E^T 1] in one
  PSUM accumulation).  exp has no max-subtraction: scores are bounded
  (~[-3.3, 3.3]) for this problem's scale.
- Output: pY [65, 1024] (64 y rows + den row) is copied PSUM->SBUF on DVE
  and DMAd per (head, q-half) as y_d[4, 65, 2048]; the host transposes and
  divides by den.  No on-device transposes.
"""

import sys

sys.path.insert(0, "/opt/trn_rl_repo")

import numpy as np

import concourse.bacc as bacc
import concourse.bass as bass
import concourse.mybir as mybir
import concourse.tile as tile
from concourse.bass import ts

P = 128
T = 2048
D = 1024
NH = 4          # heads per core
HD = 64         # head dim
TB = T // P     # 16 t-blocks
CB = D // P     # 8 c-blocks
QKV_COLS = 3 * NH * HD  # 768 per core
F32 = mybir.dt.float32
F16 = mybir.dt.float16

_CACHED = {}


def build_bass(finalize=True):
    nc = bacc.Bacc()

    # x, t-block major: row (tb, p) holds x[t in block tb][c in cb] as
    # [cb, 128] -> per-partition 2KB contiguous lines
    xtb_d = nc.dram_tensor("xtb", [TB * P, D], F16, kind="ExternalInput")
    # w, column-group major: [p, cb*col] per group, groups ordered
    # [v(2048) | q0(1024) | q1(1024) | k0(1024) | k1(1024)]
    w_d = nc.dram_tensor("w", [P, CB * QKV_COLS], F16, kind="ExternalInput")
    bqk_d = nc.dram_tensor("bqk", [P, 4], F32, kind="ExternalInput")
    bv_d = nc.dram_tensor("bv", [1, NH * HD], F32, kind="ExternalInput")
    # per head: rows 0..63 = y^T (unnormalized), row 64 = softmax denominator
    y_d = nc.dram_tensor("y", [NH, HD + 1, T], F32, kind="ExternalOutput")

    WV_O = 0                      # wv at w_d[:, 0:2048]
    WCT_O = [2048, 3072, 4096, 5120]  # ct0..ct3 offsets (q0, q1, k0, k1)

    with tile.TileContext(nc) as tc:
        with (
            tc.tile_pool(name="persist", bufs=1) as persist,
            tc.tile_pool(name="ystg", bufs=3) as ystg_pool,
            tc.tile_pool(name="epool", bufs=3) as epool,
            tc.tile_pool(name="ps_s", bufs=1, space="PSUM") as ps_s,
            tc.tile_pool(name="ps_y", bufs=1, space="PSUM") as ps_y,
        ):
            # kT: [p, t] per pair; head 2*pr at partitions 0:64, 2*pr+1 at 64:128
            kT = [persist.tile([P, T], F16, name=f"kT{i}") for i in range(2)]
            # qT: [p, t] per head, zero-padded: head h's 64 dims live at
            # partitions (h%2)*64..+64, the other 64 partitions stay zero so a
            # full-128 matmul against the kT pair tile selects only head h
            qT = [persist.tile([P, T], F16, name=f"qT{h}") for h in range(NH)]
            for h in range(NH):
                nc.vector.memset(qT[h][:], 0.0)
            # V' with ones column per head: [t-part, h, 65], one tile per tb
            vv = [
                persist.tile([P, NH, HD + 1], F16, name=f"vv{tb}")
                for tb in range(TB)
            ]
            for tb in range(TB):
                nc.vector.memset(vv[tb][:, :, HD : HD + 1], 1.0)
            bqk_sb = persist.tile([P, 4], F32)
            bvb = persist.tile([P, NH * HD], F32)

            nc.sync.dma_start(out=bqk_sb[:], in_=bqk_d[:, :])
            nc.gpsimd.dma_start(
                out=bvb[:], in_=bv_d[0:1, :].to_broadcast((P, NH * HD))
            )

            # x: [p, tb, cb, t]; per t-block DMA is contiguous on both sides
            xT = persist.tile([P, TB, CB, P], F16, name="xT")
            wv = persist.tile([P, CB, NH * HD], F16)
            wct = [
                persist.tile([P, CB, P], F16, name=f"wct{i}") for i in range(4)
            ]

            def dma_x(tb):
                for hh in range(2):
                    nc.sync.dma_start(
                        out=xT[ts(hh, 64), tb],
                        in_=xtb_d[tb * P + hh * 64 : tb * P + (hh + 1) * 64, :]
                        .rearrange("p (cb t) -> p cb t", cb=CB),
                    )

            def dma_w(tile_, off, width):
                for hh in range(2):
                    nc.sync.dma_start(
                        out=tile_[ts(hh, 64)],
                        in_=w_d[ts(hh, 64), off : off + width]
                        .rearrange("p (cb t) -> p cb t", cb=CB),
                    )

            # order: v-path first (v_proj leads), then pair-0 q/k, x blocks
            dma_w(wv, WV_O, 2048)
            dma_x(0)
            dma_x(1)
            dma_w(wct[0], WCT_O[0], 1024)
            dma_x(2)
            dma_w(wct[2], WCT_O[2], 1024)
            dma_x(3)
            dma_x(4)
            dma_x(5)
            dma_w(wct[1], WCT_O[1], 1024)
            dma_w(wct[3], WCT_O[3], 1024)
            for tb in range(6, TB):
                dma_x(tb)

            # ---------------- QKV projection --------------------------------
            # Pre-attention groups alternate S0/S1; in-attention fillers pin S1.
            s_flip = [0]

            def next_s_tag():
                s_flip[0] ^= 1
                return f"S{s_flip[0]}"

            def qk_proj(ct, chunk, tag=None):
                # one 512-wide t-chunk of q/k column group ct
                pqk = ps_s.tile(
                    [P, 512], F32, tag=tag or next_s_tag(), name="pqk"
                )
                for cb in range(CB):
                    nc.tensor.matmul(
                        pqk[:],
                        lhsT=wct[ct][:, cb, :],
                        rhs=xT[:, 4 * chunk : 4 * chunk + 4, cb, :],
                        start=(cb == 0),
                        stop=(cb == CB - 1),
                    )
                if ct < 2:
                    for s in range(2):
                        nc.vector.tensor_scalar_add(
                            out=qT[2 * ct + s][
                                s * 64 : (s + 1) * 64, ts(chunk, 512)
                            ],
                            in0=pqk[s * 64 : (s + 1) * 64, :],
                            scalar1=bqk_sb[s * 64 : (s + 1) * 64, ct : ct + 1],
                        )
                else:
                    nc.vector.tensor_scalar_add(
                        out=kT[ct - 2][:, ts(chunk, 512)],
                        in0=pqk[:],
                        scalar1=bqk_sb[:, ct : ct + 1],
                    )

            def v_proj(tb, tag=None):
                pv = ps_s.tile(
                    [P, NH * HD], F32, tag=tag or next_s_tag(), name="pv"
                )
                for cb in range(CB):
                    nc.tensor.matmul(
                        pv[:],
                        lhsT=xT[:, tb, cb, :],
                        rhs=wv[:, cb, :],
                        start=(cb == 0),
                        stop=(cb == CB - 1),
                    )
                nc.vector.tensor_tensor(
                    out=vv[tb][:, :, 0:HD],
                    in0=pv[:].rearrange("p (a b) -> p a b", a=NH),
                    in1=bvb[:].rearrange("p (a b) -> p a b", a=NH),
                    op=mybir.AluOpType.add,
                )

            # ---------------- attention -------------------------------------
            # Per kb stage: [filler?, AV(kb-1), scores(s1), exp(s1),
            # scores(s0), exp(s0)].  scores(kb) -> exp(kb) on ACT staggered
            # across the two S psum slots; the PE runs AV(kb-1) meanwhile.
            def attention(pr, qh, fillers=()):
                fillers = list(fillers)
                pY = [
                    ps_y.tile([HD + 1, 1024], F32, tag=f"Y{s}", name=f"pY{s}")
                    for s in range(2)
                ]

                def issue_av(kb, eprev):
                    for s in range(2):
                        for i in range(2):
                            nc.tensor.matmul(
                                pY[s][:, ts(i, 512)],
                                lhsT=vv[kb][:, 2 * pr + s, :],
                                rhs=eprev[s][:, ts(i, 512)],
                                start=(kb == 0),
                                stop=(kb == TB - 1),
                            )

                def scores(kb, s):
                    pS = ps_s.tile([P, 1024], F32, tag=f"S{s}", name=f"pS{s}")
                    for i in range(2):
                        nc.tensor.matmul(
                            pS[:, ts(i, 512)],
                            lhsT=kT[pr][:, ts(kb, P)],
                            rhs=qT[2 * pr + s][
                                :,
                                qh * 1024 + i * 512 : qh * 1024 + (i + 1) * 512,
                            ],
                            start=True,
                            stop=True,
                        )
                    eT = epool.tile([P, 1024], F16, tag=f"E{s}", name=f"eT{s}")
                    nc.scalar.activation(
                        out=eT[:],
                        in_=pS[:],
                        func=mybir.ActivationFunctionType.Exp,
                        scale=0.125,
                    )
                    return eT

                prev = None
                for kb in range(TB):
                    if fillers:
                        fillers.pop(0)()
                    cur = [None, None]
                    if prev is not None:
                        issue_av(kb - 1, prev)
                    cur[1] = scores(kb, 1)
                    cur[0] = scores(kb, 0)
                    prev = cur
                issue_av(TB - 1, prev)
                for f in fillers:
                    f()
                for s in range(2):
                    yst = ystg_pool.tile([HD + 1, 1024], F32, name="yst")
                    nc.vector.tensor_copy(out=yst[:], in_=pY[s][:])
                    nc.sync.dma_start(
                        out=y_d[2 * pr + s, :, ts(qh, 1024)],
                        in_=yst[:],
                    )

            # startup: v_proj 0..3 paced by the x DMA, then pair-0 q/k for the
            # first q-half; remaining projection rides inside attention
            for tb in range(4):
                v_proj(tb)
            qk_proj(0, 0)
            qk_proj(2, 0)
            v_proj(4)
            qk_proj(0, 1)
            qk_proj(2, 1)
            v_proj(5)

            F = "S1"  # filler slot during attention
            attention(0, 0, fillers=[
                lambda: qk_proj(2, 2, F), lambda: qk_proj(2, 3, F),
                lambda: v_proj(6, F), lambda: v_proj(7, F),
                lambda: v_proj(8, F), lambda: v_proj(9, F),
                lambda: qk_proj(0, 2, F), lambda: qk_proj(0, 3, F),
                lambda: v_proj(10, F), lambda: v_proj(11, F),
                lambda: v_proj(12, F), lambda: v_proj(13, F),
                lambda: v_proj(14, F), lambda: v_proj(15, F),
            ])
            attention(0, 1, fillers=[
                lambda: qk_proj(3, 0, F), lambda: qk_proj(3, 1, F),
                lambda: qk_proj(1, 0, F), lambda: qk_proj(1, 1, F),
            ])
            attention(1, 0, fillers=[
                lambda: qk_proj(3, 2, F), lambda: qk_proj(3, 3, F),
                lambda: qk_proj(1, 2, F), lambda: qk_proj(1, 3, F),
            ])
            attention(1, 1)

    if finalize:
        nc.finalize()
    return nc


def _shard_inputs(x, W_qkv, b_qkv):
    """Build per-core input maps. Core c: batch c//4, head group c%4."""
    x = np.asarray(x, dtype=np.float32)
    W = np.asarray(W_qkv, dtype=np.float32)
    b = np.asarray(b_qkv, dtype=np.float32)
    bf = np.float16
    # t-block-major x: row (tb, p) = x[c = cb*128+?][t]... laid out so that
    # xtb[tb*128+p, cb*128+t] = x[t = tb*128+t][c]?  We need
    # xT[c, t] grouped as [tb, p(=c within cb? no: p = partition = c%128?)]
    # Kernel reads row (tb, p) as [cb, t]: value = xT[cb*128+p, tb*128+t].
    xtb = []
    for bi in range(2):
        xT = x[bi].T.astype(bf)  # [D, T]
        # [cb, p, tb, t] -> [tb, p, cb, t]
        a = xT.reshape(CB, P, TB, P).transpose(2, 1, 0, 3)
        xtb.append(np.ascontiguousarray(a).reshape(TB * P, D))
    in_maps = []
    for c in range(8):
        bi, hg = c // 4, c % 4
        cs = hg * 256  # column start within each of q/k/v blocks
        qw = W[:, cs : cs + 256]
        kw = W[:, D + cs : D + cs + 256]
        vw = W[:, 2 * D + cs : 2 * D + cs + 256]
        # groups: v(256 cols) | q0 | q1 | k0 | k1 (128 cols each), each
        # stored [p, cb*cols]
        def grp(wcols):
            ncols = wcols.shape[1]
            return (
                wcols.astype(bf)
                .reshape(CB, P, ncols)
                .transpose(1, 0, 2)
                .reshape(P, CB * ncols)
            )
        w_core = np.concatenate(
            [grp(vw), grp(qw[:, :128]), grp(qw[:, 128:]),
             grp(kw[:, :128]), grp(kw[:, 128:])],
            axis=1,
        )
        bqk = np.concatenate([b[cs : cs + 256], b[D + cs : D + cs + 256]])
        bqk = np.ascontiguousarray(bqk.reshape(4, 128).T)
        bv = np.ascontiguousarray(b[2 * D + cs : 2 * D + cs + 256].reshape(1, 256))
        in_maps.append(
            {
                "xtb": xtb[bi],
                "w": np.ascontiguousarray(w_core),
                "bqk": bqk,
                "bv": bv,
            }
        )
    return in_maps


def kernel(x, W_qkv, b_qkv, trace=False):
    from concourse.bass_utils import run_bass_kernel_spmd

    if "nc" not in _CACHED:
        _CACHED["nc"] = build_bass()
    nc = _CACHED["nc"]

    in_maps = _shard_inputs(x, W_qkv, b_qkv)
    res = run_bass_kernel_spmd(nc, in_maps, list(range(8)), trace=trace)
    _CACHED["last_result"] = res

    out = np.empty((2, T, D), dtype=np.float32)
    for c in range(8):
        bi, hg = c // 4, c % 4
        yr = res.results[c]["y"]  # [NH, 65, T]
        den = yr[:, HD, :]  # [NH, T]
        y = yr[:, :HD, :] / den[:, None, :]  # [NH, HD, T]
        out[bi, :, hg * 256 : (hg + 1) * 256] = y.transpose(2, 0, 1).reshape(
            T, NH * HD
        )
    return out


if __name__ == "__main__":
    nc = build_bass()
    print("built ok")
